# revision 33
# baseline (speedup 1.0000x reference)
"""LSTMCell on 8 Trainium2 NeuronCores, data-parallel over the batch.

Full inputs: x/h_t/c_t [65536,128] f32, 8 gate weight matrices [128,128],
4 biases [128]. Returns (h_new, c_new) as [65536,128] f32 each.

Transposed-layout design: the host pre-transposes x/h_t/c_t to
[128 features, batch] fp16 so features sit on SBUF partitions and every
DMA descriptor is a large contiguous per-partition block. Weights are the
matmul stationary operand (out = W @ x^T with gate units on partitions),
so there are no PE transposes and no PSUM->SBUF cast.

Per core (8192 batch cols): supergroups (small first/last ones to
shorten pipeline fill/drain), split into groups that each fill one
[128, 4*G] PSUM quad with banks [i | f | o | 2*g_a] (W_g,b_g pre-scaled
by 2 on the host).
  - ONE sigmoid over the quad gives i,f,o and s=sigmoid(2g_a);
    tanh(g_a) = 2s-1. Sigmoid and Tanh live in the same ACT table-set,
    so tanh(c_new) is computed directly with zero table switches; a
    dummy activation preloads the table during the DMA fill.
  - DVE ops (fp16 2x/4x modes): fc = f*c; g2 = 2s-1; ig = i*g2;
    cn = ig + fc; hn = o*tanh(cn).
  - Each supergroup's tail (tanh, hn, stores) is emitted AFTER the next
    supergroup's first group so the ACT queue never blocks PSUM-buffer
    recycling (keeps the tensor engine continuously busy -> full
    p-state clock). The last supergroup processes its tail per group
    to shorten the drain.
  - Outputs written fp16, host converts back to f32 and re-transposes.
"""
import numpy as np
from contextlib import ExitStack

import concourse.bass as bass
import concourse.tile as tile
from concourse import bacc, mybir
from concourse.bass_utils import run_bass_kernel_spmd

F32 = mybir.dt.float32
F16 = mybir.dt.float16
AF = mybir.ActivationFunctionType
ALU = mybir.AluOpType

NCORES = 8
BC = 8192            # batch cols per core (transposed layout)

# (start, size, group sizes): small first sg primes the pipeline fast,
# small last sg shortens the drain.
SGS = [
    (0, 1152, (128, 512, 512)),
    (1152, 2048, (512, 512, 512, 512)),
    (3200, 2048, (512, 512, 512, 512)),
    (5248, 2048, (512, 512, 512, 512)),
    (7296, 896, (512, 256, 128)),
]

_CACHE = {}


def _build(has_bias: bool):
    nc = bacc.Bacc("TRN2", target_bir_lowering=False, debug=False)
    xt = nc.dram_tensor("xt", [128, BC], F16, kind="ExternalInput").ap()
    ht = nc.dram_tensor("ht", [128, BC], F16, kind="ExternalInput").ap()
    ct = nc.dram_tensor("ct", [128, BC], F16, kind="ExternalInput").ap()
    wxh = nc.dram_tensor("wxh", [128, 1024], F16, kind="ExternalInput").ap()
    if has_bias:
        bias = nc.dram_tensor("bias", [128, 4], F32, kind="ExternalInput").ap()
    hnt = nc.dram_tensor("hnt", [128, BC], F16, kind="ExternalOutput").ap()
    cnt = nc.dram_tensor("cnt", [128, BC], F16, kind="ExternalOutput").ap()

    nlast = len(SGS) - 1

    with tile.TileContext(nc) as tc:
        with ExitStack() as ctx:
            const = ctx.enter_context(tc.tile_pool(name="const", bufs=1))
            inp = ctx.enter_context(tc.tile_pool(name="inp", bufs=4))
            qp = ctx.enter_context(tc.tile_pool(name="qp", bufs=2, space="PSUM"))
            sp = ctx.enter_context(tc.tile_pool(name="sp", bufs=8))
            tmp = ctx.enter_context(tc.tile_pool(name="tmp", bufs=3))
            s2p = ctx.enter_context(tc.tile_pool(name="s2p", bufs=2))
            op = ctx.enter_context(tc.tile_pool(name="op", bufs=2))

            # weights first on sync (one merged transfer: fewer kicks,
            # bigger descriptors): they gate the very first ldweights
            wxh_sb = const.tile([128, 1024], F16)
            nc.sync.dma_start(wxh_sb[:], wxh)
            wx_sb = wxh_sb[:, 0:512]
            wh_sb = wxh_sb[:, 512:1024]
            if has_bias:
                b_sb = const.tile([128, 4], F32)
                nc.sync.dma_start(b_sb[:], bias)

            # Preload the sigmoid ACT table during the DMA fill so the first
            # real activation doesn't pay the ~2.7us table load.
            dummy = const.tile([128, 1], F32)
            nc.vector.memset(dummy[:], 0.0)
            dummy2 = const.tile([128, 1], F32)
            nc.scalar.activation(dummy2[:], dummy[:], AF.Sigmoid)


            def emit_sigmoid(sig, quad, gc):
                if has_bias:
                    for g in range(4):
                        nc.scalar.activation(
                            sig[:, g * gc:(g + 1) * gc],
                            quad[:, g * gc:(g + 1) * gc],
                            AF.Sigmoid, bias=b_sb[:, g:g + 1])
                else:
                    nc.scalar.activation(sig[:], quad[:], AF.Sigmoid)

            pending = None
            for si, (c0, sgc, gsizes) in enumerate(SGS):
                x_sg = inp.tile([128, sgc], F16, name=f"x{si}", tag="xg")
                h_sg = inp.tile([128, sgc], F16, name=f"h{si}", tag="hg")
                c_sg = inp.tile([128, sgc], F16, name=f"c{si}", tag="cg")
                if si == 0:
                    # split so group-0 matmuls start as soon as the first
                    # 128 cols land; critical pieces kicked before the bulk
                    g0sz = gsizes[0]
                    nc.gpsimd.dma_start(x_sg[:, 0:g0sz], xt[:, c0:c0 + g0sz])
                    nc.gpsimd.dma_start(h_sg[:, 0:g0sz], ht[:, c0:c0 + g0sz])
                    nc.sync.dma_start(x_sg[:, g0sz:sgc],
                                      xt[:, c0 + g0sz:c0 + sgc])
                    nc.sync.dma_start(h_sg[:, g0sz:sgc],
                                      ht[:, c0 + g0sz:c0 + sgc])
                    nc.sync.dma_start(c_sg[:], ct[:, c0:c0 + sgc])
                else:
                    nc.sync.dma_start(x_sg[:], xt[:, c0:c0 + sgc])
                    nc.sync.dma_start(h_sg[:], ht[:, c0:c0 + sgc])
                    nc.sync.dma_start(c_sg[:], ct[:, c0:c0 + sgc])
                cn_sg = op.tile([128, sgc], F16, name=f"cn{si}", tag="cn")
                hn_sg = op.tile([128, sgc], F16, name=f"hn{si}", tag="hn")

                sigs = []
                g0 = 0
                for t, gc in enumerate(gsizes):
                    quad = qp.tile([128, 4 * gc], F32, name=f"q{si}_{t}",
                                   tag="quad")
                    xg = x_sg[:, g0:g0 + gc]
                    hg = h_sg[:, g0:g0 + gc]
                    for g in range(4):
                        col = g * gc
                        nc.tensor.matmul(quad[:, col:col + gc],
                                         wx_sb[:, g * 128:(g + 1) * 128], xg,
                                         start=True, stop=False)
                        nc.tensor.matmul(quad[:, col:col + gc],
                                         wh_sb[:, g * 128:(g + 1) * 128], hg,
                                         start=False, stop=True)
                    sig = sp.tile([128, 4 * gc], F16, name=f"sig{si}_{t}",
                                  tag="sig")
                    emit_sigmoid(sig, quad, gc)
                    sigs.append((sig, g0, gc))

                    i_ap = sig[:, 0:gc]
                    f_ap = sig[:, gc:2 * gc]
                    s_ap = sig[:, 3 * gc:4 * gc]
                    cg = c_sg[:, g0:g0 + gc]
                    fc = tmp.tile([128, gc], F16, name=f"fc{si}_{t}", tag="fc")
                    nc.vector.tensor_mul(fc[:], f_ap, cg)
                    g2 = tmp.tile([128, gc], F16, name=f"g2{si}_{t}", tag="g2")
                    nc.vector.tensor_scalar(g2[:], s_ap, 2.0, 1.0,
                                            ALU.mult, ALU.subtract)
                    ig = tmp.tile([128, gc], F16, name=f"ig{si}_{t}", tag="ig")
                    nc.vector.tensor_mul(ig[:], i_ap, g2[:])
                    nc.vector.tensor_add(cn_sg[:, g0:g0 + gc], ig[:], fc[:])

                    if si == nlast:
                        # last supergroup: per-group tail for a short drain
                        tc = s2p.tile([128, gc], F16, name=f"tc{si}_{t}",
                                      tag="s2")
                        nc.scalar.activation(tc[:], cn_sg[:, g0:g0 + gc],
                                             AF.Tanh)
                        nc.vector.tensor_mul(hn_sg[:, g0:g0 + gc],
                                             sig[:, 2 * gc:3 * gc], tc[:])
                        if t == 0:
                            nc.sync.dma_start(cnt[:, c0:c0 + gc],
                                              cn_sg[:, 0:gc])
                        elif t == len(gsizes) - 1:
                            # batch the rest into single kicks so the final
                            # store isn't stuck behind serialized kick issue
                            p0 = g0 - sum(gsizes[1:-1])
                            nc.sync.dma_start(cnt[:, c0 + p0:c0 + sgc],
                                              cn_sg[:, p0:sgc])
                            nc.sync.dma_start(hnt[:, c0:c0 + sgc],
                                              hn_sg[:, 0:sgc])

                    if t == 0 and pending is not None:
                        pending()
                        pending = None
                    g0 += gc

                if si != nlast:
                    def mk_tail(si=si, c0=c0, sgc=sgc, cn_sg=cn_sg,
                                hn_sg=hn_sg, sigs=tuple(sigs)):
                        tc = s2p.tile([128, sgc], F16, name=f"tc_{si}",
                                      tag="tcsg")
                        nc.scalar.activation(tc[:], cn_sg[:], AF.Tanh)
                        for sig, g0, gc in sigs:
                            nc.vector.tensor_mul(
                                hn_sg[:, g0:g0 + gc],
                                sig[:, 2 * gc:3 * gc], tc[:, g0:g0 + gc])
                        # store in ~halves, split ONLY at group boundaries
                        # (a split inside a group races with its writes)
                        edges = [g0 for _, g0, _ in sigs] + [sgc]
                        hs = min(edges[1:-1], key=lambda e: abs(e - sgc // 2),
                                 default=sgc)
                        for t0, t1 in ((0, hs), (hs, sgc)):
                            if t1 <= t0:
                                continue
                            nc.sync.dma_start(
                                cnt[:, c0 + t0:c0 + t1],
                                cn_sg[:, t0:t1])
                            nc.sync.dma_start(
                                hnt[:, c0 + t0:c0 + t1],
                                hn_sg[:, t0:t1])
                    pending = mk_tail
            assert pending is None
    nc.compile()
    return nc


def _run(inputs, trace=False, tmpdir=None):
    x = np.asarray(inputs["x"], dtype=np.float32)
    h = np.asarray(inputs["h_t"], dtype=np.float32)
    c = np.asarray(inputs["c_t"], dtype=np.float32)
    # gate order [i, f, o, g]; W_g/b_g scaled by 2 for the tanh-via-sigmoid
    wx = np.concatenate([inputs["W_ii"], inputs["W_if"], inputs["W_io"],
                         2.0 * np.asarray(inputs["W_ig"])], axis=0)
    wh = np.concatenate([inputs["W_hi"], inputs["W_hf"], inputs["W_ho"],
                         2.0 * np.asarray(inputs["W_hg"])], axis=0)
    b = np.concatenate([inputs["b_i"], inputs["b_f"], inputs["b_o"],
                        2.0 * np.asarray(inputs["b_g"])], axis=0)
    wxh = np.ascontiguousarray(
        np.concatenate([wx.T, wh.T], axis=1), dtype=np.float16)  # [128,1024]
    has_bias = bool(np.any(b))

    key = has_bias
    if key not in _CACHE:
        _CACHE[key] = _build(has_bias)
    nc = _CACHE[key]

    in_maps = []
    for i in range(NCORES):
        sl = slice(i * BC, (i + 1) * BC)
        m = {
            "xt": np.ascontiguousarray(x[sl].T, dtype=np.float16),
            "ht": np.ascontiguousarray(h[sl].T, dtype=np.float16),
            "ct": np.ascontiguousarray(c[sl].T, dtype=np.float16),
            "wxh": wxh,
        }
        if has_bias:
            m["bias"] = np.ascontiguousarray(
                b.reshape(4, 128).T, dtype=np.float32)
        in_maps.append(m)

    res = run_bass_kernel_spmd(nc, in_maps, core_ids=list(range(NCORES)),
                               trace=trace, tmpdir=tmpdir)
    h_new = np.empty((NCORES * BC, 128), dtype=np.float32)
    c_new = np.empty((NCORES * BC, 128), dtype=np.float32)
    for i, r in enumerate(res.results):
        sl = slice(i * BC, (i + 1) * BC)
        h_new[sl] = r["hnt"].T.astype(np.float32)
        c_new[sl] = r["cnt"].T.astype(np.float32)
    return h_new, c_new, res


def kernel(**inputs):
    h_new, c_new, _ = _run(inputs, trace=False)
    return h_new, c_new
